# revision 1
# baseline (speedup 1.0000x reference)
"""Causal multi-head attention block (B=16, S=1024, d=1024, H=16) on 8 NeuronCores.

Strategy: data-parallel over batch (2 batches per core), no collectives.
Per-core kernel (fp16 matmuls, fp32 accumulation):
  phase A: transpose x -> xT[d, m] via PE transposes (cast fp32->fp16)
  phase B: QT = Wq @ xT, KT = Wk @ xT (transposed layout [d_out, m]),
           V  = x @ Wv.T (natural layout [m, d_out], packed in 65-wide
           per-head strips with a fused ones column)
  phase C: per (batch, head, q-chunk): scoresT[k, q] = KT.T @ QT on PE,
           exp((s + mask)/8) on ACT (no max subtraction: |s/8| is O(5)),
           causal mask via 0/1 triangle multiply on diagonal blocks +
           skipping fully-masked blocks, then out_unT[dh, q] (+ sum row,
           from the ones column) = [V|1].T @ expT accumulated on PE.
           Normalize with DVE reciprocal + gpsimd partition_broadcast.
  phase D: y = attn_outT.T @ WoT (natural layout) -> DRAM.
Biases: bq/bk are zero by problem spec (ignored); bv/bo folded in exactly
on the host (y += bv @ Wo.T + bo).
"""

import numpy as np

_CACHE: dict = {}

S = 1024
D = 1024
H = 16
DH = 64
BPC = 2           # batches per core
M = BPC * S       # tokens per core
NCORES = 8


def _build_nc():
    import concourse.bass as bass  # noqa: F401
    import concourse.mybir as mybir
    import concourse.tile as tile
    from concourse import bacc
    from concourse.masks import make_identity
    from contextlib import ExitStack

    f32 = mybir.dt.float32
    f16 = mybir.dt.float16
    EXPF = mybir.ActivationFunctionType.Exp

    nc = bacc.Bacc("TRN2", target_bir_lowering=False, debug=False,
                   num_devices=NCORES)

    x_d = nc.dram_tensor("x", [M, D], f32, kind="ExternalInput")
    wq_d = nc.dram_tensor("Wq", [D, D], f32, kind="ExternalInput")
    wk_d = nc.dram_tensor("Wk", [D, D], f32, kind="ExternalInput")
    wv_d = nc.dram_tensor("Wv", [D, D], f32, kind="ExternalInput")
    wo_d = nc.dram_tensor("Wo", [D, D], f32, kind="ExternalInput")
    tri_d = nc.dram_tensor("tri01", [128, 128], f16, kind="ExternalInput")
    y_d = nc.dram_tensor("y", [M, D], f32, kind="ExternalOutput")

    NMT = M // 128        # 16 m-tiles
    NDT = D // 128        # 8 d-tiles
    NMC = M // 512        # 4 m-chunks
    NOC = D // 512        # 2 o-chunks

    with tile.TileContext(nc) as tc, ExitStack() as top:
        consts = top.enter_context(tc.tile_pool(name="consts", bufs=1))
        persist = top.enter_context(tc.tile_pool(name="persist", bufs=1))
        wrot = top.enter_context(tc.tile_pool(name="wrot", bufs=1))

        ident = consts.tile([128, 128], f32, tag="ident")
        make_identity(nc, ident)
        tri01 = consts.tile([128, 128], f16, tag="tri")
        nc.sync.dma_start(out=tri01, in_=tri_d[:, :])

        # persistent activations (fp16)
        QT = persist.tile([128, NDT, M], f16, tag="QT")    # [o, m] transposed
        KT = persist.tile([128, NDT, M], f16, tag="KT")
        V = persist.tile([128, NMT, H * 65], f16, tag="V")  # [m, head strips]
        AO = persist.tile([128, NDT, M], f16, tag="AO")    # attn out, transposed

        def load_transposed(dst, dram, ncols, stage, psT):
            """dst[:, i_tile, c*128:(c+1)*128] = dram[c*128:(c+1)*128, :].T
            dst: [128, NDT, ncols] fp16; dram: [ncols, D] fp32."""
            for rt in range(ncols // 128):
                st = stage.tile([128, D], f32, tag="stage")
                nc.sync.dma_start(out=st, in_=dram[rt * 128:(rt + 1) * 128, :])
                for g in range(NDT // 4):
                    pt = psT.tile([128, 512], f32, tag="psT")
                    for c in range(4):
                        ib = g * 4 + c
                        nc.tensor.transpose(
                            pt[:, c * 128:(c + 1) * 128],
                            st[:, ib * 128:(ib + 1) * 128], ident)
                    nc.scalar.copy(
                        out=dst[:, g * 4:g * 4 + 4, rt * 128:(rt + 1) * 128],
                        in_=pt.rearrange("p (a b) -> p a b", b=128))

        # ---------- phases A+B: projections ----------
        with ExitStack() as ph1:
            xTp = ph1.enter_context(tc.tile_pool(name="xTp", bufs=1))
            stage = ph1.enter_context(tc.tile_pool(name="stage", bufs=2))
            psT = ph1.enter_context(tc.tile_pool(name="psT", bufs=2, space="PSUM"))
            psP = ph1.enter_context(tc.tile_pool(name="psP", bufs=4, space="PSUM"))

            xT = xTp.tile([128, NDT, M], f16, tag="xT")
            load_transposed(xT, x_d, M, stage, psT)

            # Q and K projections -> transposed layout
            for w_dram, dst in ((wq_d, QT), (wk_d, KT)):
                WT = wrot.tile([128, NDT, D], f16, tag="WT")
                load_transposed(WT, w_dram, D, stage, psT)
                for ot in range(NDT):
                    for mc in range(NMC):
                        pp = psP.tile([128, 512], f32, tag="psP")
                        for it in range(NDT):
                            nc.tensor.matmul(
                                pp,
                                WT[:, it, ot * 128:(ot + 1) * 128],
                                xT[:, it, mc * 512:(mc + 1) * 512],
                                start=(it == 0), stop=(it == NDT - 1))
                        nc.scalar.copy(
                            out=dst[:, ot, mc * 512:(mc + 1) * 512], in_=pp)

            # V projection -> natural layout in 65-wide head strips:
            # every head h: [V(64) | ones] at cols h*65..h*65+64
            WT = wrot.tile([128, NDT, D], f16, tag="WT")
            load_transposed(WT, wv_d, D, stage, psT)
            for mt in range(NMT):
                v2 = V[:, mt, :].rearrange("p (a c) -> p a c", c=65)
                nc.gpsimd.memset(v2[:, :, 64], 1.0)
                for oc in range(NOC):
                    pp = psP.tile([128, 512], f32, tag="psP")
                    for it in range(NDT):
                        nc.tensor.matmul(
                            pp,
                            xT[:, it, mt * 128:(mt + 1) * 128],
                            WT[:, it, oc * 512:(oc + 1) * 512],
                            start=(it == 0), stop=(it == NDT - 1))
                    nc.scalar.copy(
                        out=v2[:, 8 * oc:8 * oc + 8, 0:64],
                        in_=pp.rearrange("p (a c) -> p a c", c=64))

        # ---------- phase C: attention ----------
        LNF = mybir.ActivationFunctionType.Ln
        with ExitStack() as ph2:
            expp = ph2.enter_context(tc.tile_pool(name="expp", bufs=12))
            recp = ph2.enter_context(tc.tile_pool(name="recp", bufs=3))
            rbp = ph2.enter_context(tc.tile_pool(name="rbp", bufs=3))
            tmpp = ph2.enter_context(tc.tile_pool(name="tmpp", bufs=3))
            psS = ph2.enter_context(tc.tile_pool(name="psS", bufs=4, space="PSUM"))
            psO = ph2.enter_context(tc.tile_pool(name="psO", bufs=3, space="PSUM"))

            for b in range(BPC):
                for h in range(H):
                    thq = h // 2
                    po = (h % 2) * 64     # partition offset of this head
                    even = (h % 2 == 0)
                    for qc in range(2):
                        q0 = b * S + qc * 512     # global q start (m coords)
                        ps_o = psO.tile([128, 512], f32, tag="psO")
                        nkt = (qc + 1) * 4
                        for kt in range(nkt):
                            k0 = kt * 128
                            off = max(0, k0 - qc * 512)
                            kg = b * S + k0
                            ps_s = psS.tile([128, 512], f32, tag="psS")
                            nc.tensor.matmul(
                                ps_s[:, off:512],
                                KT[po:po + 64, thq, kg:kg + 128],
                                QT[po:po + 64, thq, q0 + off:q0 + 512],
                                start=True, stop=True)
                            ex = expp.tile([128, 512], f16, tag="exp")
                            nc.scalar.activation(
                                out=ex[:, off:512], in_=ps_s[:, off:512],
                                func=EXPF, scale=0.125)
                            if k0 >= qc * 512:  # diagonal block: 0/1 triangle
                                nc.vector.tensor_mul(
                                    ex[:, off:off + 128],
                                    ex[:, off:off + 128], tri01)
                            mtv = b * (S // 128) + kt
                            nc.tensor.matmul(
                                ps_o[0:65, off:512],
                                V[:, mtv, h * 65:h * 65 + 65],
                                ex[:, off:512],
                                start=(kt == 0), stop=(kt == nkt - 1))
                        # normalize: 1/sums as exp(-ln(sums)) on ACT (the
                        # 1-lane DVE reciprocal costs 3.4us; two ACT LUT
                        # ops are ~3x cheaper and run on a lighter engine)
                        rec = recp.tile([128, 512], f32, tag="rec")
                        nc.scalar.activation(out=rec[64:65, :],
                                             in_=ps_o[64:65, :], func=LNF)
                        nc.scalar.activation(out=rec[64:65, :],
                                             in_=rec[64:65, :], func=EXPF,
                                             scale=-1.0)
                        rb = rbp.tile([64, 512], f32, tag="rb")
                        r1 = rec[64:65, :]
                        rsrc = bass.AP(tensor=r1.tensor, offset=r1.offset,
                                       ap=[list(r1.ap[0]), [0, 64]]
                                       + [list(a) for a in r1.ap[1:]])
                        nc.sync.dma_start(out=rb[0:64, :], in_=rsrc)
                        if even:
                            nc.vector.tensor_mul(
                                out=AO[0:64, thq, q0:q0 + 512],
                                in0=ps_o[0:64, :], in1=rb[0:64, :])
                        else:
                            tmp = tmpp.tile([64, 512], f16, tag="tmp")
                            nc.vector.tensor_mul(
                                out=tmp, in0=ps_o[0:64, :], in1=rb[0:64, :])
                            nc.sync.dma_start(
                                out=AO[64:128, thq, q0:q0 + 512], in_=tmp)

        # ---------- phase D: output projection ----------
        with ExitStack() as ph3:
            stage = ph3.enter_context(tc.tile_pool(name="stage2", bufs=2))
            ystage = ph3.enter_context(tc.tile_pool(name="ystage", bufs=3))
            psT = ph3.enter_context(tc.tile_pool(name="psT2", bufs=2, space="PSUM"))
            psY = ph3.enter_context(tc.tile_pool(name="psY", bufs=4, space="PSUM"))

            WoT = wrot.tile([128, NDT, D], f16, tag="WT")
            load_transposed(WoT, wo_d, D, stage, psT)
            for mt in range(NMT):
                ys = ystage.tile([128, D], f32, tag="ys")
                for oc in range(NOC):
                    pp = psY.tile([128, 512], f32, tag="psY")
                    for dt_ in range(NDT):
                        nc.tensor.matmul(
                            pp,
                            AO[:, dt_, mt * 128:(mt + 1) * 128],
                            WoT[:, dt_, oc * 512:(oc + 1) * 512],
                            start=(dt_ == 0), stop=(dt_ == NDT - 1))
                    nc.scalar.copy(out=ys[:, oc * 512:(oc + 1) * 512], in_=pp)
                nc.sync.dma_start(out=y_d[mt * 128:(mt + 1) * 128, :], in_=ys)

    nc.compile()
    return nc


def _tri01():
    # tri01[dk, dq] = 1 where k <= q (allowed), else 0
    return np.triu(np.ones((128, 128), np.float16))


def _get_nc():
    if "nc" not in _CACHE:
        _CACHE["nc"] = _build_nc()
    return _CACHE["nc"]


def kernel(x, Wq, bq, Wk, bk, Wv, bv, Wo, bo):
    from concourse.bass_utils import run_bass_kernel_spmd

    x = np.ascontiguousarray(np.asarray(x, dtype=np.float32))
    B = x.shape[0]
    assert x.shape == (B, S, D) and B == NCORES * BPC
    Wq = np.ascontiguousarray(np.asarray(Wq, dtype=np.float32))
    Wk = np.ascontiguousarray(np.asarray(Wk, dtype=np.float32))
    Wv = np.ascontiguousarray(np.asarray(Wv, dtype=np.float32))
    Wo = np.ascontiguousarray(np.asarray(Wo, dtype=np.float32))

    nc = _get_nc()
    shards = x.reshape(NCORES, M, D)
    tri = _tri01()
    in_maps = [
        {"x": shards[c], "Wq": Wq, "Wk": Wk, "Wv": Wv, "Wo": Wo, "tri01": tri}
        for c in range(NCORES)
    ]
    res = run_bass_kernel_spmd(nc, in_maps, core_ids=list(range(NCORES)))
    y = np.stack([res.results[c]["y"] for c in range(NCORES)])
    y = y.reshape(B, S, D)

    # exact host-side fold of bv and bo (bq/bk are zero by problem spec)
    bias = (np.asarray(bv, np.float32) @ np.asarray(Wo, np.float32).T
            + np.asarray(bo, np.float32))
    if np.any(bias):
        y = y + bias
    return y.astype(np.float32)



# revision 5
# speedup vs baseline: 1.4253x; 1.4253x over previous
"""Causal multi-head attention block (B=16, S=1024, d=1024, H=16) on 8 NeuronCores.

Strategy: data-parallel over batch (2 batches per core), no collectives.
Host side: x shards and all four weights are transposed + cast to fp16 in
numpy, so the device does zero transposes and zero load-time casts.

Per-core schedule (4 windows, engines specialized to avoid ACT table swaps
and keep the PE out of low p-states):
  W1: QT = WqT.T @ xT and KT = WkT.T @ xT for both batches, V(b0)
      (PE dense ~137us; PSUM->SBUF copies on DVE).
  W2: attention(b0) interleaved with V(b1) projection groups + WoT load.
  W3: attention(b1) interleaved with output projection of b0.
  W4: output projection of b1 (PSUM -> DRAM directly).
Attention inner loop per (batch, head, 512-wide q-chunk): scoresT = KT.T@QT
on PE, exp((s)/8) on ACT (the ONLY op type ACT ever runs -> one table load
total), causal 0/1 triangle multiply on GPSIMD (SBUF-only engine), attnV
accumulate on PE with a fused ones column producing row sums, then
normalize: DVE reciprocal_approx_fast on the sums row, DMA partition
broadcast, DVE multiply into AO.
Biases: bq/bk are zero by problem spec (bk would cancel in softmax anyway);
bv/bo folded in exactly on the host (y += bv @ Wo.T + bo).
"""

import numpy as np

_CACHE: dict = {}

S = 1024
D = 1024
H = 16
DH = 64
BPC = 2           # batches per core
M = BPC * S       # tokens per core
NCORES = 8
NDT = D // 128    # 8 d-tiles


def _build_nc():
    import concourse.bass as bass  # noqa: F401
    import concourse.mybir as mybir
    import concourse.tile as tile
    from concourse import bacc
    from contextlib import ExitStack

    f32 = mybir.dt.float32
    f16 = mybir.dt.float16
    EXPF = mybir.ActivationFunctionType.Exp

    nc = bacc.Bacc("TRN2", target_bir_lowering=False, debug=False,
                   num_devices=NCORES)

    xT_d = nc.dram_tensor("xT", [D, M], f16, kind="ExternalInput")
    wq_d = nc.dram_tensor("WqT", [D, D], f16, kind="ExternalInput")
    wk_d = nc.dram_tensor("WkT", [D, D], f16, kind="ExternalInput")
    wv_d = nc.dram_tensor("WvT", [D, D], f16, kind="ExternalInput")
    wo_d = nc.dram_tensor("WoT", [D, D], f16, kind="ExternalInput")
    tri_d = nc.dram_tensor("tri01", [128, 128], f16, kind="ExternalInput")
    y_d = nc.dram_tensor("y", [M, D], f32, kind="ExternalOutput")

    with tile.TileContext(nc) as tc, ExitStack() as top:
        consts = top.enter_context(tc.tile_pool(name="consts", bufs=1))
        persist = top.enter_context(tc.tile_pool(name="persist", bufs=1))
        wpool = top.enter_context(tc.tile_pool(name="wpool", bufs=1))
        expp = top.enter_context(tc.tile_pool(name="expp", bufs=8))
        recp = top.enter_context(tc.tile_pool(name="recp", bufs=2))
        rbp = top.enter_context(tc.tile_pool(name="rbp", bufs=3))
        tmpp = top.enter_context(tc.tile_pool(name="tmpp", bufs=3))
        psS = top.enter_context(tc.tile_pool(name="psS", bufs=3, space="PSUM"))
        psO = top.enter_context(tc.tile_pool(name="psO", bufs=3, space="PSUM"))

        tri01 = consts.tile([128, 128], f16, tag="tri")
        nc.sync.dma_start(out=tri01, in_=tri_d[:, :])

        # persistent activations (fp16)
        QT = persist.tile([128, NDT, M], f16, tag="QT")    # [d_out, m] transp.
        KT = persist.tile([128, NDT, M], f16, tag="KT")
        V = persist.tile([128, M // 128, H * 65], f16, tag="V")  # head strips
        AO = persist.tile([128, NDT, M], f16, tag="AO")    # attn out, transp.

        WA = wpool.tile([128, NDT, D], f16, tag="WA")
        WB = wpool.tile([128, NDT, D], f16, tag="WB")

        def load_w(dst, dram):
            for rt in range(NDT):
                nc.sync.dma_start(out=dst[:, rt, :],
                                  in_=dram[rt * 128:(rt + 1) * 128, :])

        # ---------- attention for one (batch, head) ----------
        def attn_head(b, h):
            thq = h // 2
            po = (h % 2) * 64
            even = (h % 2 == 0)
            for qc in range(2):
                q0 = b * S + qc * 512
                ps_o = psO.tile([128, 512], f32, tag="psO")
                nkt = (qc + 1) * 4
                for kt in range(nkt):
                    k0 = kt * 128
                    off = max(0, k0 - qc * 512)
                    kg = b * S + k0
                    ps_s = psS.tile([128, 512], f32, tag="psS")
                    nc.tensor.matmul(
                        ps_s[:, off:512],
                        KT[po:po + 64, thq, kg:kg + 128],
                        QT[po:po + 64, thq, q0 + off:q0 + 512],
                        start=True, stop=True)
                    ex = expp.tile([128, 512], f16, tag="ex")
                    nc.scalar.activation(
                        out=ex[:, off:512], in_=ps_s[:, off:512],
                        func=EXPF, scale=0.125)
                    if k0 >= qc * 512:  # diagonal block: 0/1 triangle
                        nc.gpsimd.tensor_mul(
                            out=ex[:, off:off + 128],
                            in0=ex[:, off:off + 128], in1=tri01)
                    mtv = b * (S // 128) + kt
                    nc.tensor.matmul(
                        ps_o[0:65, off:512],
                        V[:, mtv, h * 65:h * 65 + 65],
                        ex[:, off:512],
                        start=(kt == 0), stop=(kt == nkt - 1))
                # normalize: recip of sums row (part. 64), broadcast, mul
                # NOTE: the custom-DVE recip uop only works at partition
                # offset 0 on HW, so run it over all 128 partitions and use
                # row 64 (1/sums); other rows are junk and never read.
                rec = recp.tile([128, 512], f32, tag="rec")
                nc.vector.reciprocal_approx_fast(out=rec, in_=ps_o)
                r1 = rec[64:65, :]
                rsrc = bass.AP(tensor=r1.tensor, offset=r1.offset,
                               ap=[list(r1.ap[0]), [0, 64]]
                               + [list(a) for a in r1.ap[1:]])
                rb = rbp.tile([64, 512], f32, tag="rb")
                nc.sync.dma_start(out=rb[0:64, :], in_=rsrc)
                if even:
                    nc.vector.tensor_mul(
                        out=AO[0:64, thq, q0:q0 + 512],
                        in0=ps_o[0:64, :], in1=rb[0:64, :])
                else:
                    tmp = tmpp.tile([64, 512], f16, tag="tmp")
                    nc.vector.tensor_mul(
                        out=tmp, in0=ps_o[0:64, :], in1=rb[0:64, :])
                    nc.sync.dma_start(
                        out=AO[64:128, thq, q0:q0 + 512], in_=tmp)

        def attn_batch(b, fillers):
            for h in range(H):
                attn_head(b, h)
                if fillers and h % 2 == 1:
                    fillers.pop(0)()
            while fillers:
                fillers.pop(0)()

        # ---------- W1: Q/K projections (both batches), V(b0) ----------
        load_w(WA, wq_d)
        load_w(WB, wk_d)
        with ExitStack() as s1:
            psP = s1.enter_context(
                tc.tile_pool(name="psP", bufs=2, space="PSUM"))
            xqp = s1.enter_context(tc.tile_pool(name="xqp", bufs=2))
            xvp = s1.enter_context(tc.tile_pool(name="xvp", bufs=2))

            def qk_proj_batch(dst, Wt, b):
                for mc in range(2):
                    c0 = b * S + mc * 512
                    xq = xqp.tile([128, NDT, 512], f16, tag="xq")
                    for it in range(NDT):
                        nc.sync.dma_start(
                            out=xq[:, it, :],
                            in_=xT_d[it * 128:(it + 1) * 128, c0:c0 + 512])
                    for ot in range(NDT):
                        pp = psP.tile([128, 512], f32, tag="psP")
                        for it in range(NDT):
                            nc.tensor.matmul(
                                pp,
                                Wt[:, it, ot * 128:(ot + 1) * 128],
                                xq[:, it, :],
                                start=(it == 0), stop=(it == NDT - 1))
                        nc.vector.tensor_copy(
                            out=dst[:, ot, c0:c0 + 512], in_=pp)

            def v_group(b, mt):
                gmt = b * (S // 128) + mt
                xv = xvp.tile([128, NDT, 128], f16, tag="xv")
                for it in range(NDT):
                    nc.sync.dma_start(
                        out=xv[:, it, :],
                        in_=xT_d[it * 128:(it + 1) * 128,
                                 gmt * 128:(gmt + 1) * 128])
                v2 = V[:, gmt, :].rearrange("p (a c) -> p a c", c=65)
                nc.gpsimd.memset(v2[:, :, 64], 1.0)
                for oc in range(2):
                    pp = psP.tile([128, 512], f32, tag="psP")
                    for it in range(NDT):
                        nc.tensor.matmul(
                            pp,
                            xv[:, it, :],
                            WA[:, it, oc * 512:(oc + 1) * 512],
                            start=(it == 0), stop=(it == NDT - 1))
                    nc.vector.tensor_copy(
                        out=v2[:, 8 * oc:8 * oc + 8, 0:64],
                        in_=pp.rearrange("p (a c) -> p a c", c=64))

            qk_proj_batch(QT, WA, 0)
            qk_proj_batch(QT, WA, 1)
            qk_proj_batch(KT, WB, 0)
            qk_proj_batch(KT, WB, 1)
            load_w(WA, wv_d)          # WA slot free after Q projections
            for mt in range(S // 128):
                v_group(0, mt)

            # ---------- W2: attention(b0) || V(b1) projection ----------
            load_w(WB, wo_d)          # WB slot free after K projections
            attn_batch(0, [
                (lambda mt=mt: v_group(1, mt)) for mt in range(S // 128)])

        # ---------- W3: attention(b1) || out-projection(b0) ----------
        with ExitStack() as s2:
            psY = s2.enter_context(
                tc.tile_pool(name="psY", bufs=2, space="PSUM"))
            ysp = s2.enter_context(tc.tile_pool(name="ysp", bufs=3))

            def o_group(b, mt):
                gmt = b * (S // 128) + mt
                for oc in range(2):
                    pp = psY.tile([128, 512], f32, tag="psY")
                    for dt_ in range(NDT):
                        nc.tensor.matmul(
                            pp,
                            AO[:, dt_, gmt * 128:(gmt + 1) * 128],
                            WB[:, dt_, oc * 512:(oc + 1) * 512],
                            start=(dt_ == 0), stop=(dt_ == NDT - 1))
                    ys = ysp.tile([128, 512], f32, tag="ys")
                    nc.vector.tensor_copy(out=ys, in_=pp)
                    nc.sync.dma_start(
                        out=y_d[gmt * 128:(gmt + 1) * 128,
                                oc * 512:(oc + 1) * 512],
                        in_=ys)

            attn_batch(1, [
                (lambda mt=mt: o_group(0, mt)) for mt in range(S // 128)])

            # ---------- W4: out-projection(b1) ----------
            for mt in range(S // 128):
                o_group(1, mt)

    nc.compile()
    return nc


def _tri01():
    # tri01[dk, dq] = 1 where k <= q (allowed), else 0
    return np.triu(np.ones((128, 128), np.float16))


def _get_nc():
    if "nc" not in _CACHE:
        _CACHE["nc"] = _build_nc()
    return _CACHE["nc"]


def make_in_maps(x, Wq, Wk, Wv, Wo):
    """Host-side shard + transpose + fp16 cast. x: [16, S, D] fp32."""
    shards = np.asarray(x, np.float32).reshape(NCORES, M, D)
    tri = _tri01()
    wqT = np.ascontiguousarray(np.asarray(Wq, np.float32).T).astype(np.float16)
    wkT = np.ascontiguousarray(np.asarray(Wk, np.float32).T).astype(np.float16)
    wvT = np.ascontiguousarray(np.asarray(Wv, np.float32).T).astype(np.float16)
    woT = np.ascontiguousarray(np.asarray(Wo, np.float32).T).astype(np.float16)
    maps = []
    for c in range(NCORES):
        xT = np.ascontiguousarray(shards[c].T).astype(np.float16)
        maps.append({"xT": xT, "WqT": wqT, "WkT": wkT, "WvT": wvT,
                     "WoT": woT, "tri01": tri})
    return maps


def kernel(x, Wq, bq, Wk, bk, Wv, bv, Wo, bo):
    from concourse.bass_utils import run_bass_kernel_spmd

    x = np.asarray(x, dtype=np.float32)
    B = x.shape[0]
    assert x.shape == (B, S, D) and B == NCORES * BPC

    nc = _get_nc()
    in_maps = make_in_maps(x, Wq, Wk, Wv, Wo)
    res = run_bass_kernel_spmd(nc, in_maps, core_ids=list(range(NCORES)))
    y = np.stack([res.results[c]["y"] for c in range(NCORES)])
    y = y.reshape(B, S, D)

    # exact host-side fold of bv and bo (bq/bk are zero by problem spec;
    # bk would cancel in softmax regardless)
    bias = (np.asarray(bv, np.float32) @ np.asarray(Wo, np.float32).T
            + np.asarray(bo, np.float32))
    if np.any(bias):
        y = y + bias
    return y.astype(np.float32)


# revision 6
# speedup vs baseline: 1.6751x; 1.1752x over previous
"""Causal multi-head attention block (B=16, S=1024, d=1024, H=16) on 8 NeuronCores.

Strategy: data-parallel over batch (2 batches per core), no collectives.
Host side: x shards and all four weights are transposed + cast to fp16 in
numpy, so the device does zero transposes and zero load-time casts.

Per-core schedule (engines specialized; PE kept dense to stay in the high
DVFS p-state):
  W1: Q projection (both batches), K(b0), V(b0)  -- PE dense ~110us
  W2: attention(b0) interleaved with K(b1)+V(b1) projection groups
  W3: attention(b1) interleaved with output projection of b0
  W4: output projection of b1
Attention inner loop per (batch, head, 512-wide q-chunk) is software-
pipelined: scoresT = KT.T@QT on PE and exp(s/8) on ACT run LAG=2 k-tiles
ahead of the attn@V accumulation so the PE never stalls on the ACT engine.
ACT runs ONLY Exp (one table load total). Causal triangle mask on GPSIMD.
Normalization: DVE reciprocal_approx_fast over the whole PSUM tile (row 64
holds 1/rowsums via a fused ones column in V; the custom uop requires
partition offset 0), then a 4-way-split stride-0 DMA broadcast, then a DVE
multiply. Odd heads write via a GPSIMD partition-shift copy (partitions
0:64 -> 64:128), avoiding 64-descriptor DMA shifts.
Biases: bq/bk are zero by problem spec (bk would cancel in softmax anyway);
bv/bo folded in exactly on the host (y += bv @ Wo.T + bo).
"""

import numpy as np

_CACHE: dict = {}

S = 1024
D = 1024
H = 16
DH = 64
BPC = 2           # batches per core
M = BPC * S       # tokens per core
NCORES = 8
NDT = D // 128    # 8 d-tiles
LAG = 2           # attnV software-pipeline depth


def _build_nc():
    import concourse.bass as bass  # noqa: F401
    import concourse.mybir as mybir
    import concourse.tile as tile
    from concourse import bacc
    from contextlib import ExitStack

    f32 = mybir.dt.float32
    f16 = mybir.dt.float16
    EXPF = mybir.ActivationFunctionType.Exp

    nc = bacc.Bacc("TRN2", target_bir_lowering=False, debug=False,
                   num_devices=NCORES)

    xT_d = nc.dram_tensor("xT", [D, M], f16, kind="ExternalInput")
    wq_d = nc.dram_tensor("WqT", [D, D], f16, kind="ExternalInput")
    wk_d = nc.dram_tensor("WkT", [D, D], f16, kind="ExternalInput")
    wv_d = nc.dram_tensor("WvT", [D, D], f16, kind="ExternalInput")
    wo_d = nc.dram_tensor("WoT", [D, D], f16, kind="ExternalInput")
    tri_d = nc.dram_tensor("tri01", [128, 128], f16, kind="ExternalInput")
    y_d = nc.dram_tensor("y", [M, D], f32, kind="ExternalOutput")

    with tile.TileContext(nc) as tc, ExitStack() as top:
        consts = top.enter_context(tc.tile_pool(name="consts", bufs=1))
        persist = top.enter_context(tc.tile_pool(name="persist", bufs=1))
        wpool = top.enter_context(tc.tile_pool(name="wpool", bufs=1))
        expp = top.enter_context(tc.tile_pool(name="expp", bufs=8))
        recp = top.enter_context(tc.tile_pool(name="recp", bufs=2))
        rbp = top.enter_context(tc.tile_pool(name="rbp", bufs=3))
        tmpp = top.enter_context(tc.tile_pool(name="tmpp", bufs=3))
        psS = top.enter_context(tc.tile_pool(name="psS", bufs=3, space="PSUM"))
        psO = top.enter_context(tc.tile_pool(name="psO", bufs=3, space="PSUM"))

        tri01 = consts.tile([128, 128], f16, tag="tri")
        nc.sync.dma_start(out=tri01, in_=tri_d[:, :])

        # persistent activations (fp16)
        QT = persist.tile([128, NDT, M], f16, tag="QT")    # [d_out, m] transp.
        KT = persist.tile([128, NDT, M], f16, tag="KT")
        V = persist.tile([128, M // 128, H * 65], f16, tag="V")  # head strips
        AO = persist.tile([128, NDT, M], f16, tag="AO")    # attn out, transp.

        WA = wpool.tile([128, NDT, D], f16, tag="WA")
        WB = wpool.tile([128, NDT, D], f16, tag="WB")

        def load_w(dst, dram):
            for rt in range(NDT):
                nc.sync.dma_start(out=dst[:, rt, :],
                                  in_=dram[rt * 128:(rt + 1) * 128, :])

        # ---------- attention for one (batch, head) ----------
        def attn_head(b, h):
            thq = h // 2
            po = (h % 2) * 64
            even = (h % 2 == 0)
            for qc in range(2):
                q0 = b * S + qc * 512
                ps_o = psO.tile([128, 512], f32, tag="psO")
                nkt = (qc + 1) * 4
                pend = []

                def attnv_one():
                    kt0, ex0, off0 = pend.pop(0)
                    nc.tensor.matmul(
                        ps_o[0:65, off0:512],
                        V[:, b * 8 + kt0, h * 65:h * 65 + 65],
                        ex0[:, off0:512],
                        start=(kt0 == 0), stop=(kt0 == nkt - 1))

                for kt in range(nkt):
                    k0 = kt * 128
                    off = max(0, k0 - qc * 512)
                    kg = b * S + k0
                    ps_s = psS.tile([128, 512], f32, tag="psS")
                    nc.tensor.matmul(
                        ps_s[:, off:512],
                        KT[po:po + 64, thq, kg:kg + 128],
                        QT[po:po + 64, thq, q0 + off:q0 + 512],
                        start=True, stop=True)
                    ex = expp.tile([128, 512], f16, tag="ex")
                    nc.scalar.activation(
                        out=ex[:, off:512], in_=ps_s[:, off:512],
                        func=EXPF, scale=0.125)
                    if k0 >= qc * 512:  # diagonal block: 0/1 triangle
                        nc.gpsimd.tensor_mul(
                            out=ex[:, off:off + 128],
                            in0=ex[:, off:off + 128], in1=tri01)
                    pend.append((kt, ex, off))
                    if len(pend) > LAG:
                        attnv_one()
                while pend:
                    attnv_one()
                # normalize: recip over the full tile (custom uop needs
                # partition offset 0); row 64 = 1/sums, other rows junk.
                rec = recp.tile([128, 512], f32, tag="rec")
                nc.vector.reciprocal_approx_fast(out=rec, in_=ps_o)
                r1 = rec[64:65, :]
                rb = rbp.tile([64, 512], f32, tag="rb")
                for qt in range(4):  # 4-way split partition broadcast
                    rsrc = bass.AP(tensor=r1.tensor, offset=r1.offset,
                                   ap=[list(r1.ap[0]), [0, 16]]
                                   + [list(a) for a in r1.ap[1:]])
                    nc.sync.dma_start(out=rb[16 * qt:16 * qt + 16, :],
                                      in_=rsrc)
                if even:
                    nc.vector.tensor_mul(
                        out=AO[0:64, thq, q0:q0 + 512],
                        in0=ps_o[0:64, :], in1=rb[0:64, :])
                else:
                    tmp = tmpp.tile([64, 512], f16, tag="tmp")
                    nc.vector.tensor_mul(
                        out=tmp, in0=ps_o[0:64, :], in1=rb[0:64, :])
                    nc.gpsimd.tensor_copy(
                        out=AO[64:128, thq, q0:q0 + 512], in_=tmp)

        def attn_batch(b, fillers):
            for h in range(H):
                attn_head(b, h)
                k = (len(fillers) + (H - 1 - h)) // (H - h)
                for _ in range(min(k, len(fillers))):
                    fillers.pop(0)()
            while fillers:
                fillers.pop(0)()

        # ---------- projection group generators ----------
        with ExitStack() as s1:
            psP = s1.enter_context(
                tc.tile_pool(name="psP", bufs=2, space="PSUM"))
            xqp = s1.enter_context(tc.tile_pool(name="xqp", bufs=2))

            def load_chunk(st, c0):
                xq = xqp.tile([128, NDT, 512], f16, tag="xq")
                for it in range(NDT):
                    nc.sync.dma_start(
                        out=xq[:, it, :],
                        in_=xT_d[it * 128:(it + 1) * 128, c0:c0 + 512])
                st["xq"] = xq

            def qk_fillers(dst, Wt, b):
                """16 closures: 2 chunks x 8 ot-groups; chunk DMA with g0."""
                out = []
                for ch in range(2):
                    c0 = b * S + ch * 512
                    st = {}
                    for ot in range(NDT):
                        def g(st=st, c0=c0, ot=ot, dst=dst, Wt=Wt):
                            if ot == 0:
                                load_chunk(st, c0)
                            pp = psP.tile([128, 512], f32, tag="psP")
                            for it in range(NDT):
                                nc.tensor.matmul(
                                    pp,
                                    Wt[:, it, ot * 128:(ot + 1) * 128],
                                    st["xq"][:, it, :],
                                    start=(it == 0), stop=(it == NDT - 1))
                            nc.vector.tensor_copy(
                                out=dst[:, ot, c0:c0 + 512], in_=pp)
                        out.append(g)
                return out

            def v_fillers(b):
                """16 closures: 2 chunks x (4 sub x 2 oc) groups."""
                out = []
                for ch in range(2):
                    c0 = b * S + ch * 512
                    st = {}
                    for sub in range(4):
                        for oc in range(2):
                            def g(st=st, c0=c0, ch=ch, sub=sub, oc=oc, b=b):
                                if sub == 0 and oc == 0:
                                    load_chunk(st, c0)
                                gmt = b * 8 + ch * 4 + sub
                                v2 = V[:, gmt, :].rearrange(
                                    "p (a c) -> p a c", c=65)
                                if oc == 0:
                                    nc.gpsimd.memset(v2[:, :, 64], 1.0)
                                pp = psP.tile([128, 512], f32, tag="psP")
                                for it in range(NDT):
                                    nc.tensor.matmul(
                                        pp,
                                        st["xq"][:, it,
                                                 sub * 128:(sub + 1) * 128],
                                        WA[:, it, oc * 512:(oc + 1) * 512],
                                        start=(it == 0), stop=(it == NDT - 1))
                                nc.vector.tensor_copy(
                                    out=v2[:, 8 * oc:8 * oc + 8, 0:64],
                                    in_=pp.rearrange("p (a c) -> p a c", c=64))
                            out.append(g)
                return out

            # ---------- W1: Q(b0), Q(b1), K(b0), V(b0) ----------
            load_w(WA, wq_d)
            load_w(WB, wk_d)
            for g in qk_fillers(QT, WA, 0):
                g()
            for g in qk_fillers(QT, WA, 1):
                g()
            for g in qk_fillers(KT, WB, 0):
                g()
            load_w(WA, wv_d)          # WA free after Q projections
            for g in v_fillers(0):
                g()

            # ---------- W2: attention(b0) || K(b1), V(b1) ----------
            fillers = qk_fillers(KT, WB, 1) + v_fillers(1)
            fillers.append(lambda: load_w(WA, wo_d))  # WA free after V(b1)
            attn_batch(0, fillers)

        # ---------- W3: attention(b1) || out-projection(b0) ----------
        with ExitStack() as s2:
            psY = s2.enter_context(
                tc.tile_pool(name="psY", bufs=2, space="PSUM"))
            ysp = s2.enter_context(tc.tile_pool(name="ysp", bufs=3))

            def o_fillers(b):
                out = []
                for mt in range(8):
                    gmt = b * 8 + mt
                    for oc in range(2):
                        def g(gmt=gmt, oc=oc):
                            pp = psY.tile([128, 512], f32, tag="psY")
                            for dt_ in range(NDT):
                                nc.tensor.matmul(
                                    pp,
                                    AO[:, dt_, gmt * 128:(gmt + 1) * 128],
                                    WA[:, dt_, oc * 512:(oc + 1) * 512],
                                    start=(dt_ == 0), stop=(dt_ == NDT - 1))
                            ys = ysp.tile([128, 512], f32, tag="ys")
                            nc.vector.tensor_copy(out=ys, in_=pp)
                            nc.sync.dma_start(
                                out=y_d[gmt * 128:(gmt + 1) * 128,
                                        oc * 512:(oc + 1) * 512],
                                in_=ys)
                        out.append(g)
                return out

            attn_batch(1, o_fillers(0))

            # ---------- W4: out-projection(b1) ----------
            for g in o_fillers(1):
                g()

    nc.compile()
    return nc


def _tri01():
    # tri01[dk, dq] = 1 where k <= q (allowed), else 0
    return np.triu(np.ones((128, 128), np.float16))


def _get_nc():
    if "nc" not in _CACHE:
        _CACHE["nc"] = _build_nc()
    return _CACHE["nc"]


def make_in_maps(x, Wq, Wk, Wv, Wo):
    """Host-side shard + transpose + fp16 cast. x: [16, S, D] fp32."""
    shards = np.asarray(x, np.float32).reshape(NCORES, M, D)
    tri = _tri01()
    wqT = np.ascontiguousarray(np.asarray(Wq, np.float32).T).astype(np.float16)
    wkT = np.ascontiguousarray(np.asarray(Wk, np.float32).T).astype(np.float16)
    wvT = np.ascontiguousarray(np.asarray(Wv, np.float32).T).astype(np.float16)
    woT = np.ascontiguousarray(np.asarray(Wo, np.float32).T).astype(np.float16)
    maps = []
    for c in range(NCORES):
        xT = np.ascontiguousarray(shards[c].T).astype(np.float16)
        maps.append({"xT": xT, "WqT": wqT, "WkT": wkT, "WvT": wvT,
                     "WoT": woT, "tri01": tri})
    return maps


def kernel(x, Wq, bq, Wk, bk, Wv, bv, Wo, bo):
    from concourse.bass_utils import run_bass_kernel_spmd

    x = np.asarray(x, dtype=np.float32)
    B = x.shape[0]
    assert x.shape == (B, S, D) and B == NCORES * BPC

    nc = _get_nc()
    in_maps = make_in_maps(x, Wq, Wk, Wv, Wo)
    res = run_bass_kernel_spmd(nc, in_maps, core_ids=list(range(NCORES)))
    y = np.stack([res.results[c]["y"] for c in range(NCORES)])
    y = y.reshape(B, S, D)

    # exact host-side fold of bv and bo (bq/bk are zero by problem spec;
    # bk would cancel in softmax regardless)
    bias = (np.asarray(bv, np.float32) @ np.asarray(Wo, np.float32).T
            + np.asarray(bo, np.float32))
    if np.any(bias):
        y = y + bias
    return y.astype(np.float32)


# revision 7
# speedup vs baseline: 1.6754x; 1.0002x over previous
"""Causal multi-head attention block (B=16, S=1024, d=1024, H=16) on 8 NeuronCores.

Strategy: data-parallel over batch (2 batches per core), no collectives.
Host side: x shards and all four weights are transposed + cast to fp16 in
numpy, so the device does zero transposes and zero load-time casts.

Per-core schedule (engines specialized; PE kept dense to stay in the high
DVFS p-state):
  W1: Q projection (both batches), K(b0), V(b0)  -- PE dense ~110us
  W2: attention(b0) interleaved with K(b1)+V(b1) projection groups
  W3: attention(b1) interleaved with output projection of b0
  W4: output projection of b1
Attention inner loop per (batch, head, 512-wide q-chunk) is software-
pipelined: scoresT = KT.T@QT on PE and exp(s/8) on ACT run LAG=2 k-tiles
ahead of the attn@V accumulation so the PE never stalls on the ACT engine.
ACT runs ONLY Exp (one table load total). Causal triangle mask on GPSIMD.
Normalization: DVE reciprocal_approx_fast over the whole PSUM tile (row 64
holds 1/rowsums via a fused ones column in V; the custom uop requires
partition offset 0), then a 4-way-split stride-0 DMA broadcast, then a DVE
multiply. Odd heads write via a GPSIMD partition-shift copy (partitions
0:64 -> 64:128), avoiding 64-descriptor DMA shifts.
Biases: bq/bk are zero by problem spec (bk would cancel in softmax anyway);
bv/bo folded in exactly on the host (y += bv @ Wo.T + bo).
"""

import numpy as np

_CACHE: dict = {}

S = 1024
D = 1024
H = 16
DH = 64
BPC = 2           # batches per core
M = BPC * S       # tokens per core
NCORES = 8
NDT = D // 128    # 8 d-tiles
LAG = 3           # attnV software-pipeline depth


def _build_nc():
    import concourse.bass as bass  # noqa: F401
    import concourse.mybir as mybir
    import concourse.tile as tile
    from concourse import bacc
    from contextlib import ExitStack

    f32 = mybir.dt.float32
    f16 = mybir.dt.float16
    EXPF = mybir.ActivationFunctionType.Exp

    nc = bacc.Bacc("TRN2", target_bir_lowering=False, debug=False,
                   num_devices=NCORES)

    xT_d = nc.dram_tensor("xT", [D, M], f16, kind="ExternalInput")
    wq_d = nc.dram_tensor("WqT", [D, D], f16, kind="ExternalInput")
    wk_d = nc.dram_tensor("WkT", [D, D], f16, kind="ExternalInput")
    wv_d = nc.dram_tensor("WvT", [D, D], f16, kind="ExternalInput")
    wo_d = nc.dram_tensor("WoT", [D, D], f16, kind="ExternalInput")
    tri_d = nc.dram_tensor("tri01", [128, 128], f16, kind="ExternalInput")
    y_d = nc.dram_tensor("y", [M, D], f32, kind="ExternalOutput")

    with tile.TileContext(nc) as tc, ExitStack() as top:
        consts = top.enter_context(tc.tile_pool(name="consts", bufs=1))
        persist = top.enter_context(tc.tile_pool(name="persist", bufs=1))
        wpool = top.enter_context(tc.tile_pool(name="wpool", bufs=1))
        expp = top.enter_context(tc.tile_pool(name="expp", bufs=8))
        recp = top.enter_context(tc.tile_pool(name="recp", bufs=3))
        rbp = top.enter_context(tc.tile_pool(name="rbp", bufs=4))
        tmpp = top.enter_context(tc.tile_pool(name="tmpp", bufs=3))
        psS = top.enter_context(tc.tile_pool(name="psS", bufs=4, space="PSUM"))
        psO = top.enter_context(tc.tile_pool(name="psO", bufs=2, space="PSUM"))

        tri01 = consts.tile([128, 128], f16, tag="tri")
        nc.sync.dma_start(out=tri01, in_=tri_d[:, :])

        # persistent activations (fp16)
        QT = persist.tile([128, NDT, M], f16, tag="QT")    # [d_out, m] transp.
        KT = persist.tile([128, NDT, M], f16, tag="KT")
        V = persist.tile([128, M // 128, H * 65], f16, tag="V")  # head strips
        AO = persist.tile([128, NDT, M], f16, tag="AO")    # attn out, transp.

        WA = wpool.tile([128, NDT, D], f16, tag="WA")
        WB = wpool.tile([128, NDT, D], f16, tag="WB")

        def load_w(dst, dram):
            for rt in range(NDT):
                nc.sync.dma_start(out=dst[:, rt, :],
                                  in_=dram[rt * 128:(rt + 1) * 128, :])

        # ---------- attention for one (batch, head) ----------
        def attn_head(b, h):
            thq = h // 2
            po = (h % 2) * 64
            even = (h % 2 == 0)
            for qc in range(2):
                q0 = b * S + qc * 512
                ps_o = psO.tile([128, 512], f32, tag="psO")
                nkt = (qc + 1) * 4
                pend = []

                def attnv_one():
                    kt0, ex0, off0 = pend.pop(0)
                    nc.tensor.matmul(
                        ps_o[0:65, off0:512],
                        V[:, b * 8 + kt0, h * 65:h * 65 + 65],
                        ex0[:, off0:512],
                        start=(kt0 == 0), stop=(kt0 == nkt - 1))

                for kt in range(nkt):
                    k0 = kt * 128
                    off = max(0, k0 - qc * 512)
                    kg = b * S + k0
                    ps_s = psS.tile([128, 512], f32, tag="psS")
                    nc.tensor.matmul(
                        ps_s[:, off:512],
                        KT[po:po + 64, thq, kg:kg + 128],
                        QT[po:po + 64, thq, q0 + off:q0 + 512],
                        start=True, stop=True)
                    ex = expp.tile([128, 512], f16, tag="ex")
                    nc.scalar.activation(
                        out=ex[:, off:512], in_=ps_s[:, off:512],
                        func=EXPF, scale=0.125)
                    if k0 >= qc * 512:  # diagonal block: 0/1 triangle
                        nc.gpsimd.tensor_mul(
                            out=ex[:, off:off + 128],
                            in0=ex[:, off:off + 128], in1=tri01)
                    pend.append((kt, ex, off))
                    if len(pend) > LAG:
                        attnv_one()
                while pend:
                    attnv_one()
                # normalize: recip over the full tile (custom uop needs
                # partition offset 0); row 64 = 1/sums, other rows junk.
                rec = recp.tile([128, 512], f32, tag="rec")
                nc.vector.reciprocal_approx_fast(out=rec, in_=ps_o)
                r1 = rec[64:65, :]
                rb = rbp.tile([64, 512], f32, tag="rb")
                for qt in range(4):  # 4-way split partition broadcast
                    rsrc = bass.AP(tensor=r1.tensor, offset=r1.offset,
                                   ap=[list(r1.ap[0]), [0, 16]]
                                   + [list(a) for a in r1.ap[1:]])
                    nc.sync.dma_start(out=rb[16 * qt:16 * qt + 16, :],
                                      in_=rsrc)
                if even:
                    nc.vector.tensor_mul(
                        out=AO[0:64, thq, q0:q0 + 512],
                        in0=ps_o[0:64, :], in1=rb[0:64, :])
                else:
                    tmp = tmpp.tile([64, 512], f16, tag="tmp")
                    nc.vector.tensor_mul(
                        out=tmp, in0=ps_o[0:64, :], in1=rb[0:64, :])
                    nc.gpsimd.tensor_copy(
                        out=AO[64:128, thq, q0:q0 + 512], in_=tmp)

        def attn_batch(b, fillers):
            for h in range(H):
                attn_head(b, h)
                k = (len(fillers) + (H - 1 - h)) // (H - h)
                for _ in range(min(k, len(fillers))):
                    fillers.pop(0)()
            while fillers:
                fillers.pop(0)()

        # ---------- projection group generators ----------
        with ExitStack() as s1:
            psP = s1.enter_context(
                tc.tile_pool(name="psP", bufs=2, space="PSUM"))
            xqp = s1.enter_context(tc.tile_pool(name="xqp", bufs=2))

            def load_chunk(st, c0):
                xq = xqp.tile([128, NDT, 512], f16, tag="xq")
                for it in range(NDT):
                    nc.sync.dma_start(
                        out=xq[:, it, :],
                        in_=xT_d[it * 128:(it + 1) * 128, c0:c0 + 512])
                st["xq"] = xq

            def qk_fillers(dst, Wt, b):
                """16 closures: 2 chunks x 8 ot-groups; chunk DMA with g0."""
                out = []
                for ch in range(2):
                    c0 = b * S + ch * 512
                    st = {}
                    for ot in range(NDT):
                        def g(st=st, c0=c0, ot=ot, dst=dst, Wt=Wt):
                            if ot == 0:
                                load_chunk(st, c0)
                            pp = psP.tile([128, 512], f32, tag="psP")
                            for it in range(NDT):
                                nc.tensor.matmul(
                                    pp,
                                    Wt[:, it, ot * 128:(ot + 1) * 128],
                                    st["xq"][:, it, :],
                                    start=(it == 0), stop=(it == NDT - 1))
                            nc.vector.tensor_copy(
                                out=dst[:, ot, c0:c0 + 512], in_=pp)
                        out.append(g)
                return out

            def v_fillers(b):
                """16 closures: 2 chunks x (4 sub x 2 oc) groups."""
                out = []
                for ch in range(2):
                    c0 = b * S + ch * 512
                    st = {}
                    for sub in range(4):
                        for oc in range(2):
                            def g(st=st, c0=c0, ch=ch, sub=sub, oc=oc, b=b):
                                if sub == 0 and oc == 0:
                                    load_chunk(st, c0)
                                gmt = b * 8 + ch * 4 + sub
                                v2 = V[:, gmt, :].rearrange(
                                    "p (a c) -> p a c", c=65)
                                if oc == 0:
                                    nc.gpsimd.memset(v2[:, :, 64], 1.0)
                                pp = psP.tile([128, 512], f32, tag="psP")
                                for it in range(NDT):
                                    nc.tensor.matmul(
                                        pp,
                                        st["xq"][:, it,
                                                 sub * 128:(sub + 1) * 128],
                                        WA[:, it, oc * 512:(oc + 1) * 512],
                                        start=(it == 0), stop=(it == NDT - 1))
                                nc.vector.tensor_copy(
                                    out=v2[:, 8 * oc:8 * oc + 8, 0:64],
                                    in_=pp.rearrange("p (a c) -> p a c", c=64))
                            out.append(g)
                return out

            # ---------- W1: Q(b0), Q(b1), K(b0), V(b0) ----------
            load_w(WA, wq_d)
            load_w(WB, wk_d)
            for g in qk_fillers(QT, WA, 0):
                g()
            for g in qk_fillers(QT, WA, 1):
                g()
            for g in qk_fillers(KT, WB, 0):
                g()
            load_w(WA, wv_d)          # WA free after Q projections
            for g in v_fillers(0):
                g()

            # ---------- W2: attention(b0) || K(b1), V(b1) ----------
            fillers = qk_fillers(KT, WB, 1) + v_fillers(1)
            fillers.append(lambda: load_w(WA, wo_d))  # WA free after V(b1)
            attn_batch(0, fillers)

        # ---------- W3: attention(b1) || out-projection(b0) ----------
        with ExitStack() as s2:
            psY = s2.enter_context(
                tc.tile_pool(name="psY", bufs=2, space="PSUM"))
            ysp = s2.enter_context(tc.tile_pool(name="ysp", bufs=3))

            def o_fillers(b):
                out = []
                for mt in range(8):
                    gmt = b * 8 + mt
                    for oc in range(2):
                        def g(gmt=gmt, oc=oc):
                            pp = psY.tile([128, 512], f32, tag="psY")
                            for dt_ in range(NDT):
                                nc.tensor.matmul(
                                    pp,
                                    AO[:, dt_, gmt * 128:(gmt + 1) * 128],
                                    WA[:, dt_, oc * 512:(oc + 1) * 512],
                                    start=(dt_ == 0), stop=(dt_ == NDT - 1))
                            ys = ysp.tile([128, 512], f32, tag="ys")
                            nc.vector.tensor_copy(out=ys, in_=pp)
                            nc.sync.dma_start(
                                out=y_d[gmt * 128:(gmt + 1) * 128,
                                        oc * 512:(oc + 1) * 512],
                                in_=ys)
                        out.append(g)
                return out

            attn_batch(1, o_fillers(0))

            # ---------- W4: out-projection(b1) ----------
            for g in o_fillers(1):
                g()

    nc.compile()
    return nc


def _tri01():
    # tri01[dk, dq] = 1 where k <= q (allowed), else 0
    return np.triu(np.ones((128, 128), np.float16))


def _get_nc():
    if "nc" not in _CACHE:
        _CACHE["nc"] = _build_nc()
    return _CACHE["nc"]


def make_in_maps(x, Wq, Wk, Wv, Wo):
    """Host-side shard + transpose + fp16 cast. x: [16, S, D] fp32."""
    shards = np.asarray(x, np.float32).reshape(NCORES, M, D)
    tri = _tri01()
    wqT = np.ascontiguousarray(np.asarray(Wq, np.float32).T).astype(np.float16)
    wkT = np.ascontiguousarray(np.asarray(Wk, np.float32).T).astype(np.float16)
    wvT = np.ascontiguousarray(np.asarray(Wv, np.float32).T).astype(np.float16)
    woT = np.ascontiguousarray(np.asarray(Wo, np.float32).T).astype(np.float16)
    maps = []
    for c in range(NCORES):
        xT = np.ascontiguousarray(shards[c].T).astype(np.float16)
        maps.append({"xT": xT, "WqT": wqT, "WkT": wkT, "WvT": wvT,
                     "WoT": woT, "tri01": tri})
    return maps


def kernel(x, Wq, bq, Wk, bk, Wv, bv, Wo, bo):
    from concourse.bass_utils import run_bass_kernel_spmd

    x = np.asarray(x, dtype=np.float32)
    B = x.shape[0]
    assert x.shape == (B, S, D) and B == NCORES * BPC

    nc = _get_nc()
    in_maps = make_in_maps(x, Wq, Wk, Wv, Wo)
    res = run_bass_kernel_spmd(nc, in_maps, core_ids=list(range(NCORES)))
    y = np.stack([res.results[c]["y"] for c in range(NCORES)])
    y = y.reshape(B, S, D)

    # exact host-side fold of bv and bo (bq/bk are zero by problem spec;
    # bk would cancel in softmax regardless)
    bias = (np.asarray(bv, np.float32) @ np.asarray(Wo, np.float32).T
            + np.asarray(bo, np.float32))
    if np.any(bias):
        y = y + bias
    return y.astype(np.float32)
